# revision 38
# baseline (speedup 1.0000x reference)
"""Trainium2 Bass kernel for a ViT/BEiT-style transformer block (v2).

Data-parallel over batch (64 -> 8 per core), no collectives.  Feature-major
activations [features(128-part, k-tiles), tokens(free)]; big GEMMs fp8e4m3
DoubleRow (weights x64 host-side, 1/64 descale in drains); fp32 PSUM.

v2 changes over the 400us baseline (trace-driven):
  - HAM throttle fix: the baseline spent 157us at K=4/8 half-clock during
    attention.  v2 keeps the PE array full-height: rpb identity matmuls go
    FIRST (start=True, full 128 rows, no stale PSUM reads), AV stationaries
    are 128-col (64 ones-cols + 64 v-cols), v-GEMM mt1 uses overlapped
    128-col stationaries (DoubleRow both m-tiles), LN stats use fat
    [128,128] stationaries.
  - ACT op merging (each ACT op pays ~352 fixed cycles): ONE exp per pair
    over a [128,2,400] 2-bank PSUM super-tile; ONE gelu per (hh,
    chunk-pair); merged q/k drains over 2-output-tile supers.
  - Softmax denominator rides AV as 64 replicated output rows (ones-cols
    first in the stationary) -> reciprocal runs directly on [64, 2*NP],
    no per-pair gpsimd broadcast, no ACT copy.
  - proj + LN2 stats/apply interleaved into the attention pair loop; MLP
    follows immediately (no serial LN2 phase).
  - LN squares and LN2-mul on GpSimd; LN applies as single 3D DVE ops.
  - Output stored bf16 (halves output DMA).
  - Zero-bias fast path (all biases are zero for this problem's inputs);
    host asserts and falls back to biased drains if not.

SBUF slots: xT bf16, kTt bf16 (->w2 fp8 slot reuse would deadlock the
interleave, so xT2 has its own tag), qT bf16 (-> w2 fp8), vtok bf16
[128,b,mt,h,128] (ones|v), h1 fp8 [*,KT,T+64] (overlap cols for v mt1),
aoT fp8, h2 fp8, xT2 bf16, rpb bf16.
"""

import numpy as np
import ml_dtypes
from contextlib import ExitStack

import concourse.bacc as bacc
import concourse.bass as bass
import concourse.mybir as mybir
import concourse.tile as tile
from concourse.bass_utils import run_bass_kernel_spmd

bf16 = ml_dtypes.bfloat16
dt = mybir.dt
AF = mybir.ActivationFunctionType
ALU = mybir.AluOpType
DR = mybir.MatmulPerfMode.DoubleRow

# ---- problem dims (hardcoded) ----
B, N, D, H, DH, HID = 64, 197, 768, 12, 64, 3072
NCORES = 8
BPC = B // NCORES          # 8 batch elements per core
NP = 200                   # padded tokens per batch element
T = BPC * NP               # 1600 token-columns per core
TV = T + 64                # h1 extra cols so v mt1 stationaries can read 128
KT = D // 128              # 6 feature k-tiles
HT = HID // 128            # 24 hidden tiles
NCHUNK = 4
CHUNK = T // NCHUNK        # 400
MT = 2                     # m-tiles per batch element (128 + 69)
MSZ = [128, N - 128]       # [128, 69]
HP = H // 2                # 6 head-pairs
EPS = 1e-5
FS = 1.0 / 64.0            # fp8 weight descale

_NC_CACHE = {}

import os
DEBUG = os.environ.get("KDEBUG", "0") == "1"


def _build_nc(zero_bias=True):
    key = (zero_bias, DEBUG)
    if key in _NC_CACHE:
        return _NC_CACHE[key]
    nc = bacc.Bacc(None, target_bir_lowering=False)

    # ---- DRAM I/O ----
    d_xT = nc.dram_tensor("xT", [D, T], dt.bfloat16, kind="ExternalInput")
    d_wqkv = nc.dram_tensor("wqkvT", [D, 3 * D], dt.float8e4, kind="ExternalInput")
    d_wp = nc.dram_tensor("wpT", [D, D], dt.float8e4, kind="ExternalInput")
    d_w1 = nc.dram_tensor("w1T", [D, HID], dt.float8e4, kind="ExternalInput")
    d_w2 = nc.dram_tensor("w2T", [HID, D], dt.float8e4, kind="ExternalInput")
    d_qb = nc.dram_tensor("qb", [128, KT], dt.float32, kind="ExternalInput")
    d_kb = nc.dram_tensor("kb", [128, KT], dt.float32, kind="ExternalInput")
    d_vb = nc.dram_tensor("vb", [1, D], dt.bfloat16, kind="ExternalInput")
    d_pb = nc.dram_tensor("pb", [128, KT], dt.float32, kind="ExternalInput")
    d_b1 = nc.dram_tensor("b1", [128, HT], dt.float32, kind="ExternalInput")
    d_b2 = nc.dram_tensor("b2", [128, KT], dt.float32, kind="ExternalInput")
    d_id = nc.dram_tensor("ident", [128, 128], dt.bfloat16, kind="ExternalInput")
    d_rpb = nc.dram_tensor("rpbT", [128, H, MT * NP], dt.bfloat16, kind="ExternalInput")
    d_yT = nc.dram_tensor("yT", [D, T], dt.bfloat16, kind="ExternalOutput")
    if DEBUG:
        d_dh1 = nc.dram_tensor("dh1", [128, KT * TV], dt.float8e4,
                               kind="ExternalOutput")
        d_dq = nc.dram_tensor("dq", [128, KT, T], dt.bfloat16,
                              kind="ExternalOutput")
        d_dk = nc.dram_tensor("dk", [128, KT, T], dt.bfloat16,
                              kind="ExternalOutput")
        d_dao = nc.dram_tensor("dao", [128, KT, T], dt.float8e4,
                               kind="ExternalOutput")
        d_dx2 = nc.dram_tensor("dx2", [128, KT, T], dt.bfloat16,
                               kind="ExternalOutput")
        d_dpb = nc.dram_tensor("dpb", [128, 2, MT * NP], dt.bfloat16,
                               kind="ExternalOutput")
        d_dav = nc.dram_tensor("dav", [128, 2 * NP], dt.float32,
                               kind="ExternalOutput")
        d_drc = nc.dram_tensor("drc", [64, NP], dt.float32,
                               kind="ExternalOutput")
        d_drc2 = nc.dram_tensor("drc2", [64, NP], dt.float32,
                                kind="ExternalOutput")
        d_dvt = nc.dram_tensor("dvt", [128, BPC, MT, HP * 192], dt.bfloat16,
                               kind="ExternalOutput")

    with ExitStack() as ctx:
        tc = ctx.enter_context(tile.TileContext(nc))

        p_const = tc.alloc_tile_pool(name="const", bufs=1)
        p_rows = tc.alloc_tile_pool(name="prows", bufs=2)
        p_big = tc.alloc_tile_pool(name="pbig", bufs=1)

        # constants
        ones_mu = p_const.tile([128, 128], dt.bfloat16)   # col0 = 1/D, rest 0
        ones_sq = p_const.tile([128, 128], dt.bfloat16)   # col0 = 1,   rest 0
        nc.vector.memset(ones_mu[:], 0.0)
        nc.vector.memset(ones_sq[:], 0.0)
        nc.vector.memset(ones_mu[:, 0:1], 1.0 / D)
        nc.vector.memset(ones_sq[:, 0:1], 1.0 / D)
        eps_t = p_const.tile([1, 1], dt.float32)
        nc.vector.memset(eps_t[:], EPS)
        t_qb = p_const.tile([128, KT], dt.float32)
        t_kb = p_const.tile([128, KT], dt.float32)
        t_vb = p_const.tile([1, D], dt.bfloat16)
        t_pb = p_const.tile([128, KT], dt.float32)
        t_b1 = p_const.tile([128, HT], dt.float32)
        t_b2 = p_const.tile([128, KT], dt.float32)
        t_id = p_const.tile([128, 128], dt.bfloat16)

        # long-lived tiles
        xT = p_big.tile([128, KT, T], dt.bfloat16, tag="tg_x")
        xTr = d_xT.rearrange("(k p) t -> p k t", p=128)
        kTt = p_big.tile([128, KT, T], dt.bfloat16, tag="tg_k")
        qT = p_big.tile([128, KT, T], dt.bfloat16, tag="tg_q")
        # v token-major, sandwich layout per (b, mt, head-pair jj):
        # [v_{2jj} (64) | ones (64) | v_{2jj+1} (64)] -- each head's AV
        # stationary is a contiguous 128-col slice sharing the ones block:
        # even heads [v|ones] put the softmax denominator on AV out rows
        # 64:128, odd heads [ones|v] put it on rows 0:64.  mt1 rows 69:128
        # zeroed (pad m-rows contribute 0 to AV and denominator).
        vtok = p_big.tile([128, BPC, MT, HP * 192], dt.bfloat16, tag="tg_v")

        def vaug(b, mt, h):
            base = 192 * (h // 2) + 64 * (h % 2)
            return vtok[0:128, b, mt, base:base + 128]
        # h1 lives in an 18KB slot sized for the w1 weights that reuse it
        h1big = p_big.tile([128, KT * HID], dt.float8e4, tag="tg_h1")
        h1 = h1big[:, 0:KT * TV].rearrange("p (k t) -> p k t", k=KT)
        rpb = p_big.tile([128, H, MT * NP], dt.bfloat16, tag="tg_rpb")
        xT2 = p_big.tile([128, KT, T], dt.bfloat16, tag="tg_x2")
        aoT = p_big.tile([128, KT, T], dt.float8e4, tag="tg_ao")
        h2 = p_big.tile([128, KT, T], dt.float8e4, tag="tg_h2")

        # vtok ones / zero pattern (pad m-rows = 0 so they add nothing to
        # AV or the denominator; exp(garbage) rows multiply these zeros).
        # Partition base must be 0/32/64/96: zero rows 64:128 first, then
        # overwrite the valid-row ones blocks (rows 64:69 regain their 1s).
        vt_r = vtok[:].rearrange("p b mt (jj s) -> p b mt jj s", s=192)
        nc.vector.memset(vt_r[64:128, :, 1, :, :], 0.0)
        nc.vector.memset(vt_r[:, :, 0, :, 64:128], 1.0)
        nc.vector.memset(vt_r[0:MSZ[1], :, 1, :, 64:128], 1.0)

        # sync-queue DMA order: x c0, consts, wqkv-qk, x c1..c3, wqkv-v,
        # rpb, wp, w1, (post-B) w2, y stores.
        nc.sync.dma_start(xT[:, :, bass.ts(0, CHUNK)], xTr[:, :, bass.ts(0, CHUNK)])
        for t_, d_ in [(t_qb, d_qb), (t_kb, d_kb), (t_vb, d_vb), (t_pb, d_pb),
                       (t_b1, d_b1), (t_b2, d_b2), (t_id, d_id)]:
            nc.sync.dma_start(t_[:], d_[:])
        p_qkvw = tc.alloc_tile_pool(name="pqkvw", bufs=1)
        wqkv = p_qkvw.tile([128, KT, 3 * D], dt.float8e4)
        wqkvr = d_wqkv.rearrange("(k p) m -> p k m", p=128)
        nc.sync.dma_start(wqkv[:, :, 0:2 * D], wqkvr[:, :, 0:2 * D])
        for c in range(1, NCHUNK):
            cs = bass.ts(c, CHUNK)
            nc.sync.dma_start(xT[:, :, cs], xTr[:, :, cs])
        nc.sync.dma_start(wqkv[:, :, 2 * D:3 * D], wqkvr[:, :, 2 * D:3 * D])
        nc.sync.dma_start(rpb[:], d_rpb[:])

        # v-bias broadcast to all partitions (feature-varying row; biased
        # path only -- the zero-bias path drains v with a plain scale)
        if not zero_bias:
            vb_full = p_const.tile([128, D], dt.bfloat16)
            nc.gpsimd.partition_broadcast(vb_full[:], t_vb[:])

        # ============ LayerNorm pieces (feature-major) ============
        # squares on GpSimd (one 3D op per chunk); stats via fat matmuls
        # ([128,128] stationary, out rows 1:128 zero) into one 2-bank super.
        def ln_square(tmp_pool, src_bf, c, tagp=""):
            cs = bass.ts(c, CHUNK)
            x2 = tmp_pool.tile([128, KT, CHUNK], dt.bfloat16, tag="x2" + tagp, bufs=1)
            nc.gpsimd.tensor_mul(x2[:], src_bf[:, :, cs], src_bf[:, :, cs])
            return x2

        def ln_stats_fat(psum_pool, src_bf, c, x2):
            cs = bass.ts(c, CHUNK)
            st = psum_pool.tile([128, 2, 512], dt.float32, tag="stat", bufs=1)
            for k in range(KT):
                nc.tensor.matmul(st[:, 0, 0:CHUNK], ones_mu[:], src_bf[:, k, cs],
                                 start=(k == 0), stop=(k == KT - 1))
                nc.tensor.matmul(st[:, 1, 0:CHUNK], ones_sq[:], x2[:, k, :],
                                 start=(k == 0), stop=(k == KT - 1))
            return _ln_rows(st[0:1, 0, 0:CHUNK], st[0:1, 1, 0:CHUNK])

        def ln_stats_thin(psum_pool, src_bf, c, x2):
            cs = bass.ts(c, CHUNK)
            st = psum_pool.tile([128, 512], dt.float32, tag="stat2", bufs=1)
            for k in range(KT):
                nc.tensor.matmul(st[0:1, 0:CHUNK], ones_mu[:, 0:1], src_bf[:, k, cs],
                                 start=(k == 0), stop=(k == KT - 1))
                nc.tensor.matmul(st[32:33, 0:CHUNK], ones_sq[:, 0:1], x2[:, k, :],
                                 start=(k == 0), stop=(k == KT - 1))
            return _ln_rows(st[0:1, 0:CHUNK], st[32:33, 0:CHUNK])

        def _ln_rows(mu_ap, ms_ap):
            musq = p_rows.tile([1, CHUNK], dt.float32, tag="musq", bufs=1)
            nc.scalar.square(musq[:], mu_ap)
            var = p_rows.tile([1, CHUNK], dt.float32, tag="var", bufs=1)
            nc.vector.tensor_sub(var[:], ms_ap, musq[:])
            std = p_rows.tile([1, CHUNK], dt.float32, tag="std", bufs=1)
            nc.scalar.activation(std[:], var[:], AF.Sqrt, bias=eps_t[0:1, 0:1])
            a_f = p_rows.tile([1, CHUNK], dt.float32, tag="af")
            nc.vector.reciprocal_approx_fast(a_f[:], std[:])
            b_f = p_rows.tile([1, CHUNK], dt.float32, tag="bf")
            nc.vector.scalar_tensor_tensor(b_f[:], mu_ap, -1.0, a_f[:],
                                           op0=ALU.mult, op1=ALU.mult)
            return a_f, b_f

        def ln_bcast(tmp_pool, a_f, b_f, tagp=""):
            bc_a = tmp_pool.tile([128, CHUNK], dt.bfloat16, tag="bca" + tagp, bufs=1)
            bc_b = tmp_pool.tile([128, CHUNK], dt.bfloat16, tag="bcb" + tagp, bufs=1)
            a_b = p_rows.tile([1, CHUNK], dt.bfloat16, tag="afb", bufs=1)
            b_b = p_rows.tile([1, CHUNK], dt.bfloat16, tag="bfb", bufs=1)
            with nc.allow_low_precision(reason="ln rows bf16"):
                nc.vector.tensor_copy(a_b[:], a_f[:])
                nc.vector.tensor_copy(b_b[:], b_f[:])
            nc.gpsimd.partition_broadcast(bc_a[:], a_b[:])
            nc.gpsimd.partition_broadcast(bc_b[:], b_b[:])
            return bc_a, bc_b

        def ln_apply(tmp_pool, src_bf, dst_f8, c, bc_a, bc_b, mul_pool=False,
                     tagp=""):
            cs = bass.ts(c, CHUNK)
            tmp = tmp_pool.tile([128, KT, CHUNK], dt.bfloat16, tag="ntmp" + tagp,
                                bufs=1)
            for k in range(KT):
                if mul_pool:
                    nc.gpsimd.tensor_mul(tmp[:, k, :], src_bf[:, k, cs], bc_a[:])
                else:
                    nc.vector.tensor_mul(tmp[:, k, :], src_bf[:, k, cs], bc_a[:])
                with nc.allow_low_precision(reason="ln out fp8"):
                    nc.vector.tensor_add(dst_f8[:, k, cs], tmp[:, k, :], bc_b[:])

        # ============ Phase A: LN1 + QKV + V (1-chunk pipeline) ============
        p_atmp = tc.alloc_tile_pool(name="patmp", bufs=1)
        psA = tc.alloc_tile_pool(name="psA", bufs=1, space="PSUM")

        def qk_supers(c):
            cs = bass.ts(c, CHUNK)
            for which, dst, tb in ((0, qT, t_qb), (1, kTt, t_kb)):
                base = which * D
                for s in range(KT // 2):
                    pq = psA.tile([128, 2, 512], dt.float32, tag="qk", bufs=2)
                    for sub in range(2):
                        d_i = 2 * s + sub
                        for kp in range(KT // 2):
                            nc.tensor.matmul(
                                pq[:, sub, 0:CHUNK],
                                wqkv[:, 2 * kp:2 * kp + 2,
                                     base + d_i * 128:base + d_i * 128 + 128],
                                h1[:, 2 * kp:2 * kp + 2, cs],
                                start=(kp == 0), stop=(kp == KT // 2 - 1),
                                perf_mode=DR)
                    if zero_bias:
                        nc.scalar.activation(dst[:, 2 * s:2 * s + 2, cs],
                                             pq[:, :, 0:CHUNK], AF.Identity,
                                             scale=FS)
                    else:
                        for sub in range(2):
                            nc.scalar.activation(
                                dst[:, 2 * s + sub, cs], pq[:, sub, 0:CHUNK],
                                AF.Identity, bias=tb[:, 2 * s + sub:2 * s + sub + 1],
                                scale=FS)

        def v_chunk(c):
            for b in (2 * c, 2 * c + 1):
                for mt in range(MT):
                    msz = MSZ[mt]
                    n0 = b * NP + mt * 128
                    for half in range(2):
                        pv = psA.tile([128, 384], dt.float32, tag="pv", bufs=2)
                        for kp in range(KT // 2):
                            nc.tensor.matmul(
                                pv[:],
                                h1[:, 2 * kp:2 * kp + 2, n0:n0 + 128],
                                wqkv[:, 2 * kp:2 * kp + 2,
                                     2 * D + half * 384:2 * D + half * 384 + 384],
                                start=(kp == 0), stop=(kp == KT // 2 - 1),
                                perf_mode=DR)
                        # heads 6*half..6*half+5 -> v blocks at 192*jj (+128
                        # for odd): 4D out AP [p, jj(3)@192, e(2)@128, w(64)]
                        vdst = (vtok[0:msz, b, mt,
                                     576 * half:576 * (half + 1)]
                                .rearrange("p (jj a w) -> p jj a w", a=3, w=64)
                                [:, :, 0:3:2, :])
                        pvr = pv[0:msz, :].rearrange("p (jj e w) -> p jj e w",
                                                     e=2, w=64)
                        with nc.allow_low_precision(reason="v bf16 store"):
                            if zero_bias:
                                nc.vector.tensor_scalar_mul(vdst, pvr, FS)
                            else:
                                nc.vector.scalar_tensor_tensor(
                                    vdst, pvr, FS,
                                    vb_full[0:msz,
                                            half * 384:(half + 1) * 384]
                                    .rearrange("p (jj e w) -> p jj e w",
                                               e=2, w=64),
                                    op0=ALU.mult, op1=ALU.add)

        lnA = {}
        for c in range(NCHUNK + 1):
            if c >= 1:
                a_f, b_f = lnA.pop(c - 1)
                bc_a, bc_b = ln_bcast(p_atmp, a_f, b_f)
                ln_apply(p_atmp, xT, h1, c - 1, bc_a, bc_b)
                qk_supers(c - 1)
            if c < NCHUNK:
                x2 = ln_square(p_atmp, xT, c)
                lnA[c] = ln_stats_fat(psA, xT, c, x2)
            if c >= 1:
                v_chunk(c - 1)

        psA.release()
        p_atmp.release()
        p_qkvw.release()

        # wp prefetch; w1 reuses the dead h1 slot (lands mid-B, needed at D)
        p_dw = tc.alloc_tile_pool(name="pdw", bufs=1)
        wp = p_dw.tile([128, KT, D], dt.float8e4)
        nc.sync.dma_start(wp[:], d_wp.rearrange("(k p) m -> p k m", p=128))
        w1 = p_big.tile([128, KT, HID], dt.float8e4, tag="tg_h1")
        nc.sync.dma_start(w1[:], d_w1.rearrange("(k p) m -> p k m", p=128))

        # ============ Phase B: attention + interleaved proj + LN2 ==========
        p_aw = tc.alloc_tile_pool(name="paw", bufs=2)
        p_btmp = tc.alloc_tile_pool(name="pbtmp", bufs=1)
        psB = tc.alloc_tile_pool(name="psB", bufs=1, space="PSUM")

        PAIRS = [(b, j) for b in range(BPC) for j in range(HP)]
        NPAIR = len(PAIRS)
        state = {}

        def stage0(p):                 # rpb (full-height, first) + qk scores
            b, j = p
            ts_n = slice(b * NP, (b + 1) * NP)
            sc = psB.tile([128, 2, 512], dt.float32, tag="sc", bufs=2,
                          name=f"sc_{b}_{j}")
            for e in range(2):
                nc.tensor.matmul(sc[:, e, 0:MT * NP], t_id[:],
                                 rpb[:, 2 * j + e, :], start=True, stop=False)
                for mt in range(MT):
                    msz = MSZ[mt]
                    m0 = b * NP + mt * 128
                    hp0 = 64 * e
                    nc.tensor.matmul(sc[0:msz, e, bass.ts(mt, NP)],
                                     kTt[hp0:hp0 + 64, j, m0:m0 + msz],
                                     qT[hp0:hp0 + 64, j, ts_n],
                                     start=False, stop=(mt == MT - 1))
            state[p] = {"sc": sc}

        def stage1(p):                 # ONE merged exp (ACT)
            b, j = p
            st = state[p]
            p_bf = p_aw.tile([128, 2, MT * NP], dt.bfloat16, tag="pbf",
                             name=f"pbf_{b}_{j}")
            nc.scalar.activation(p_bf[:], st["sc"][:, :, 0:MT * NP], AF.Exp)
            if DEBUG and p == (0, 0):
                nc.sync.dma_start(d_dpb[:], p_bf[:])
            st["p_bf"] = p_bf

        def stage2(p):                 # AV + denom-x64 (PE, full-height)
            b, j = p
            st = state[p]
            av = psB.tile([128, 2 * NP], dt.float32, tag="av", bufs=2,
                          name=f"av_{b}_{j}")
            for e in range(2):
                h = 2 * j + e
                for mt in range(MT):
                    nc.tensor.matmul(av[:, e * NP:(e + 1) * NP],
                                     vaug(b, mt, h),
                                     st["p_bf"][:, e, bass.ts(mt, NP)],
                                     start=(mt == 0), stop=(mt == MT - 1))
            if DEBUG and p == (0, 0):
                avc = p_aw.tile([128, 2 * NP], dt.float32, tag="avc", bufs=1)
                nc.vector.tensor_copy(avc[:], av[:])
                nc.sync.dma_start(d_dav[:], avc[:])
            st["av"] = av

        def stage3(p):                 # recip denom rows, normalize AV rows
            b, j = p
            ts_n = slice(b * NP, (b + 1) * NP)
            st = state[p]
            av = st["av"]
            # e=0 ([v|ones]): AV rows 0:64, denom rows 64:128
            # e=1 ([ones|v]): denom rows 0:64, AV rows 64:128
            # reciprocal_approx_fast needs base partition 0 (in AND out);
            # cross-base tensor_copy moves e0's denom down first.
            rc = p_aw.tile([64, 2 * NP], dt.float32, tag="rc", name=f"rc_{b}_{j}")
            dcp = p_aw.tile([64, NP], dt.float32, tag="dcp", name=f"dc_{b}_{j}")
            nc.vector.tensor_copy(dcp[:], av[64:128, 0:NP])
            nc.vector.reciprocal_approx_fast(rc[0:64, 0:NP], dcp[:])
            nc.vector.reciprocal_approx_fast(rc[0:64, NP:2 * NP],
                                             av[0:64, NP:2 * NP])
            if DEBUG and p == (0, 0):
                nc.sync.dma_start(d_drc[:], rc[0:64, 0:NP])
                nc.sync.dma_start(d_drc2[:], rc[0:64, NP:2 * NP])
            with nc.allow_low_precision(reason="attn out fp8"):
                nc.vector.tensor_mul(aoT[0:64, j, ts_n],
                                     av[0:64, 0:NP], rc[0:64, 0:NP])
                nc.vector.tensor_mul(aoT[64:128, j, ts_n],
                                     av[64:128, NP:2 * NP], rc[0:64, NP:2 * NP])
            del state[p]

        def proj_super(c, s):
            cs = bass.ts(c, CHUNK)
            pp = psB.tile([128, 2, 512], dt.float32, tag="sc", bufs=2,
                          name=f"pp_{c}_{s}")
            for sub in range(2):
                d_i = 2 * s + sub
                for kp in range(KT // 2):
                    nc.tensor.matmul(pp[:, sub, 0:CHUNK],
                                     wp[:, 2 * kp:2 * kp + 2, bass.ts(d_i, 128)],
                                     aoT[:, 2 * kp:2 * kp + 2, cs],
                                     start=(kp == 0), stop=(kp == KT // 2 - 1),
                                     perf_mode=DR)
            if zero_bias:
                ptmp = p_btmp.tile([128, 2, CHUNK], dt.bfloat16, tag="ptmp",
                                   bufs=2)
                with nc.allow_low_precision(reason="residual bf16"):
                    nc.scalar.activation(ptmp[:], pp[:, :, 0:CHUNK], AF.Identity,
                                         scale=FS)
                    nc.vector.tensor_add(xT2[:, 2 * s:2 * s + 2, cs], ptmp[:],
                                         xT[:, 2 * s:2 * s + 2, cs])
            else:
                for sub in range(2):
                    d_i = 2 * s + sub
                    ptmp = p_btmp.tile([128, CHUNK], dt.bfloat16, tag="ptmp1",
                                       bufs=2)
                    with nc.allow_low_precision(reason="residual bf16"):
                        nc.scalar.activation(ptmp[:], pp[:, sub, 0:CHUNK],
                                             AF.Identity,
                                             bias=t_pb[:, d_i:d_i + 1], scale=FS)
                        nc.vector.tensor_add(xT2[:, d_i, cs], ptmp[:],
                                             xT[:, d_i, cs])

        lnB = {}

        def b_insert(t):
            # t = completed-pair index; chunk c pairs end at t == 12c+11
            if t < 11:
                return
            c, r = divmod(t - 11, 12)
            if c >= NCHUNK:
                return
            if r == 0:
                proj_super(c, 0)
            elif r == 1:
                proj_super(c, 1)
            elif r == 2:
                proj_super(c, 2)
            elif r == 3:
                x2 = ln_square(p_btmp, xT2, c, tagp="b")
                lnB[c] = ln_stats_thin(psB, xT2, c, x2)
            elif r == 4:
                a_f, b_f = lnB.pop(c)
                bc_a, bc_b = ln_bcast(p_btmp, a_f, b_f, tagp="b")
                ln_apply(p_btmp, xT2, h2, c, bc_a, bc_b, mul_pool=True,
                         tagp="b")

        for i in range(NPAIR + 2):
            if i - 2 >= 0:
                stage3(PAIRS[i - 2])
                b_insert(i - 2)
            if 0 <= i - 1 < NPAIR:
                stage2(PAIRS[i - 1])
            if i < NPAIR:
                stage0(PAIRS[i])
                stage1(PAIRS[i])
        for t in range(NPAIR, NPAIR + 17):
            b_insert(t)

        if DEBUG:
            nc.sync.dma_start(d_dh1[:], h1big[:, 0:KT * TV])
            nc.sync.dma_start(d_dq[:], qT[:])
            nc.sync.dma_start(d_dk[:], kTt[:])
            nc.sync.dma_start(d_dao[:], aoT[:])
            nc.sync.dma_start(d_dx2[:], xT2[:])
            nc.sync.dma_start(d_dvt[:], vtok[:])

        psB.release()
        p_btmp.release()
        p_aw.release()
        p_dw.release()

        # w2 prefetch into the qT slot (free after the last stage0)
        w2 = p_big.tile([128, HT, D], dt.float8e4, tag="tg_q")
        nc.sync.dma_start(w2[:], d_w2.rearrange("(k p) m -> p k m", p=128))

        # ============ Phase D: MLP (2 chunk-pairs) ============
        p_g = tc.alloc_tile_pool(name="pg", bufs=1)
        p_y = tc.alloc_tile_pool(name="py", bufs=2)
        psD = tc.alloc_tile_pool(name="psD", bufs=1, space="PSUM")

        def mlp_pair(P):
            c0, c1 = 2 * P, 2 * P + 1
            cps = [bass.ts(c0, CHUNK), bass.ts(c1, CHUNK)]
            g = p_g.tile([128, 2, HT, CHUNK], dt.float8e4, tag="g")
            for hh in range(HT):
                pfs = psD.tile([128, 2, 512], dt.float32, tag="f1", bufs=2,
                               name=f"pf_{P}_{hh}")
                for kp in range(KT // 2):
                    for i in range(2):
                        nc.tensor.matmul(
                            pfs[:, i, 0:CHUNK],
                            w1[:, 2 * kp:2 * kp + 2, bass.ts(hh, 128)],
                            h2[:, 2 * kp:2 * kp + 2, cps[i]],
                            start=(kp == 0), stop=(kp == KT // 2 - 1),
                            perf_mode=DR)
                with nc.allow_low_precision(reason="gelu fp8"):
                    nc.scalar.activation(g[:, :, hh, :], pfs[:, :, 0:CHUNK],
                                         AF.Gelu, bias=t_b1[:, hh:hh + 1],
                                         scale=FS)
            for d_i in range(KT):
                pos = psD.tile([128, 2, 512], dt.float32, tag="f2", bufs=2,
                               name=f"po_{P}_{d_i}")
                for hp in range(HT // 2):
                    for i in range(2):
                        nc.tensor.matmul(
                            pos[:, i, 0:CHUNK],
                            w2[:, 2 * hp:2 * hp + 2, bass.ts(d_i, 128)],
                            g[:, i, 2 * hp:2 * hp + 2, :],
                            start=(hp == 0), stop=(hp == HT // 2 - 1),
                            perf_mode=DR)
                ytmp = p_y.tile([128, 2, CHUNK], dt.bfloat16, tag="yt", bufs=2)
                with nc.allow_low_precision(reason="fc2 drain bf16"):
                    nc.scalar.activation(ytmp[:], pos[:, :, 0:CHUNK],
                                         AF.Identity, scale=FS)
                for i in range(2):
                    y = p_y.tile([128, CHUNK], dt.bfloat16, tag="y", bufs=3)
                    with nc.allow_low_precision(reason="y bf16 store"):
                        nc.vector.scalar_tensor_tensor(
                            y[:], ytmp[:, i, :], t_b2[:, d_i:d_i + 1],
                            xT2[:, d_i, cps[i]], op0=ALU.add, op1=ALU.add)
                    nc.sync.dma_start(
                        d_yT.rearrange("(k p) t -> p k t", p=128)[:, d_i, cps[i]],
                        y[:])

        for P in range(NCHUNK // 2):
            mlp_pair(P)

        psD.release()
        p_y.release()
        p_g.release()
        p_big.release()
        p_rows.release()
        p_const.release()

    nc.finalize()
    _NC_CACHE[key] = nc
    return nc


def _prep_host(inputs):
    """Fold LN affines / scales / gammas into weights; build per-core in_maps."""
    f = np.float32
    x = np.asarray(inputs["x"], f)
    n1w, n1b = np.asarray(inputs["norm1_w"], f), np.asarray(inputs["norm1_b"], f)
    n2w, n2b = np.asarray(inputs["norm2_w"], f), np.asarray(inputs["norm2_b"], f)
    qkv_w = np.asarray(inputs["qkv_w"], f)
    q_bias, v_bias = np.asarray(inputs["q_bias"], f), np.asarray(inputs["v_bias"], f)
    rpb_table = np.asarray(inputs["rpb_table"], f)
    rel_index = np.asarray(inputs["rel_index"])
    proj_w, proj_b = np.asarray(inputs["proj_w"], f), np.asarray(inputs["proj_b"], f)
    g1, g2 = np.asarray(inputs["gamma1"], f), np.asarray(inputs["gamma2"], f)
    fc1_w, fc1_b = np.asarray(inputs["fc1_w"], f), np.asarray(inputs["fc1_b"], f)
    fc2_w, fc2_b = np.asarray(inputs["fc2_w"], f), np.asarray(inputs["fc2_b"], f)

    scale = DH ** -0.5
    f8 = ml_dtypes.float8_e4m3
    WS_ = 64.0
    Wq, Wk, Wv = qkv_w[0:D], qkv_w[D:2 * D], qkv_w[2 * D:3 * D]
    WqT = (WS_ * scale * (Wq * n1w[None, :]).T).astype(f8)
    WkT = (WS_ * (Wk * n1w[None, :]).T).astype(f8)
    WvT = (WS_ * (Wv * n1w[None, :]).T).astype(f8)
    wqkvT = np.ascontiguousarray(np.concatenate([WqT, WkT, WvT], axis=1))
    qb = (scale * (Wq @ n1b + q_bias)).reshape(KT, 128).T.copy()   # [128, KT]
    kb = (Wk @ n1b).reshape(KT, 128).T.copy()
    vb = (Wv @ n1b + v_bias).reshape(1, D).astype(bf16)
    wpT = np.ascontiguousarray((WS_ * g1[:, None] * proj_w).T.astype(f8))
    pb = (g1 * proj_b).reshape(KT, 128).T.copy()
    w1T = np.ascontiguousarray((WS_ * fc1_w * n2w[None, :]).T.astype(f8))
    b1 = (fc1_w @ n2b + fc1_b).reshape(HT, 128).T.copy()
    w2T = np.ascontiguousarray((WS_ * g2[:, None] * fc2_w).T.astype(f8))
    b2 = (g2 * fc2_b).reshape(KT, 128).T.copy()

    # rpbT[p, h, mt*NP+n] = rpb[h, n, m=mt*128+p]  (scoresT orientation)
    RPB = rpb_table[rel_index]            # [n, m, H]
    rpbT = np.zeros((128, H, MT * NP), f)
    for mt in range(MT):
        msz = MSZ[mt]
        blk = RPB[:, mt * 128:mt * 128 + msz, :].transpose(1, 2, 0)  # [m_sl, H, n]
        for h in range(H):
            rpbT[0:msz, h, mt * NP:mt * NP + N] = blk[:, h, :]
    rpbT = rpbT.astype(bf16)

    zero_bias = not (np.any(qb) or np.any(kb) or np.any(pb))

    shared = dict(wqkvT=wqkvT, wpT=wpT, w1T=w1T, w2T=w2T,
                  qb=np.ascontiguousarray(qb), kb=np.ascontiguousarray(kb),
                  vb=vb, pb=np.ascontiguousarray(pb),
                  b1=np.ascontiguousarray(b1), b2=np.ascontiguousarray(b2),
                  ident=np.eye(128, dtype=bf16), rpbT=rpbT)
    in_maps = []
    for core in range(NCORES):
        xs = x[core * BPC:(core + 1) * BPC]            # [BPC, N, D]
        xp = np.zeros((BPC, NP, D), f)
        xp[:, 0:N, :] = xs
        xTc = np.ascontiguousarray(xp.reshape(T, D).T).astype(bf16)  # [D, T]
        m = dict(shared)
        m["xT"] = xTc
        in_maps.append(m)
    return in_maps, zero_bias


LAST_DEBUG = None


def kernel(**inputs) -> np.ndarray:
    global LAST_DEBUG
    in_maps, zero_bias = _prep_host(inputs)
    nc = _build_nc(zero_bias)
    res = run_bass_kernel_spmd(nc, in_maps, core_ids=list(range(NCORES)))
    if DEBUG:
        LAST_DEBUG = res.results
    outs = []
    for core in range(NCORES):
        yT = res.results[core]["yT"]                   # [D, T] bf16
        yp = np.asarray(yT, np.float32).T.reshape(BPC, NP, D)
        outs.append(yp[:, 0:N, :])
    return np.concatenate(outs, axis=0)
